# revision 6
# baseline (speedup 1.0000x reference)
"""DGCN layer (message passing GNN) on 8 Trainium2 NeuronCores via Bass/Tile.

v3. The gather data path is HBM random-read-bandwidth bound (~90 GB/s for
256B rows), so this version minimizes gathered bytes and row-read locality:
  - Single bf16 table (outdeg^-0.5 folded in). No lo/hi table split: each
    window's edges are sorted by src and cut into 1024-edge gather chunks;
    each chunk's dma_gather uses a chunk-specific table base row (shared
    across cores = min over cores), so int16 indices cover the chunk span.
    Sorted rows also improve HBM row-buffer locality.
  - Uniform T=16 tiles/window (balancer keeps every window <= 2048 edges)
    -> exactly 2 x 1024-idx gather instructions per window, 98 total.
  - sel built per window: tiles 0..T-4 in <=8-tile groups on DVE (is_equal
    + coef mult, bf16), last 3 tiles on the Scalar engine via
    Abs(iota - r) then Relu(coef - coef*t) with per-partition bias/scale.
  - 4 SWDGE queues; bf16 matmuls (FWL); interleaved phase 2.
"""

import math

import numpy as np

P = 128
ALPHA = 0.5
N_CORES = 8
GCH = 8  # tiles per dma_gather (hw limit: <=1024 idxs/inst)
ACT_T = 3  # sel tiles per window built on the scalar engine
IDX_SPAN = 32768


def _wrap_idx16(flat):
    """dma_gather index layout: entry k -> partition k%16, column k//16,
    replicated across the 8 gpsimd core groups (partitions 16-127)."""
    n = flat.shape[-1]
    assert n % 16 == 0
    cols = n // 16
    w = np.asarray(flat, np.int16).reshape(cols, 16).T  # [16, cols]
    return np.tile(w, (8, 1))  # [128, cols]


def _prep_host(h, src, dst, distance, n_cores):
    N, D = h.shape
    E = src.shape[0]
    npc = N // n_cores
    n_windows = (npc + P - 1) // P

    src = np.asarray(src).astype(np.int64)
    dst = np.asarray(dst).astype(np.int64)
    distance = np.asarray(distance)

    out_deg = np.bincount(src, minlength=N).astype(np.float64)
    in_deg = np.bincount(dst, minlength=N).astype(np.float64)
    coef_all = (np.float64(ALPHA) ** distance.astype(np.float64)).astype(np.float32)
    s_all = in_deg**-1.5

    # Deal nodes (sorted by in-degree) into n_cores*n_windows bins in rounds;
    # heaviest nodes go to the lightest bins -> per-window edge counts are
    # near-equal, so every window fits the same tile count T.
    n_bins = n_cores * n_windows
    order_nodes = np.argsort(-in_deg, kind="stable")
    node_bin = np.empty(N, np.int64)
    node_slot = np.empty(N, np.int64)
    esum = np.zeros(n_bins, np.int64)
    fill = np.zeros(n_bins, np.int64)
    pos = 0
    while pos < N:
        take = min(n_bins, N - pos)
        nodes_r = order_nodes[pos : pos + take]
        bins_r = np.argsort(esum, kind="stable")[:take]
        node_bin[nodes_r] = bins_r
        node_slot[nodes_r] = fill[bins_r]
        fill[bins_r] += 1
        esum[bins_r] += in_deg[nodes_r].astype(np.int64)
        pos += take
    node_core = node_bin // n_windows
    node_window = node_bin % n_windows

    core_of = node_core[dst]
    w_of = node_window[dst]
    r_of = node_slot[dst].astype(np.float32)

    gw = core_of * n_windows + w_of
    counts = np.bincount(gw, minlength=n_bins)
    maxc = int(counts.max())
    T = max(1, int(math.ceil(maxc / P)))
    cap = T * P
    n_cols = n_windows * T

    # sort edges by (core, window, src) in one argsort
    key = gw * (1 << 17) + src
    order = np.argsort(key, kind="stable")
    sgw = gw[order]
    win_start = np.concatenate([[0], np.cumsum(counts)[:-1]])
    q = np.arange(E, dtype=np.int64) - win_start[sgw]  # pos within window

    core_arr = sgw // n_windows
    w_arr = sgw % n_windows

    # padded per-window edge arrays [cores, windows, cap]
    srcs = np.full((n_cores, n_windows, cap), -1, np.int64)
    rofs_e = np.zeros((n_cores, n_windows, cap), np.float32)
    coef_e = np.zeros((n_cores, n_windows, cap), np.float32)
    srcs[core_arr, w_arr, q] = src[order]
    rofs_e[core_arr, w_arr, q] = r_of[order]
    coef_e[core_arr, w_arr, q] = coef_all[order]

    # chunk bases: global (min over cores) per (window, chunk). Chunks are
    # GCH tiles = GCH*P sorted positions. Position 8ch*128 is always a real
    # edge when counts > previous chunk capacity; assert that.
    n_chunks = (T + GCH - 1) // GCH
    bases = np.zeros((n_windows, n_chunks), np.int64)
    for ch in range(n_chunks):
        p0 = ch * GCH * P
        assert (counts.reshape(n_cores, n_windows) > p0).all(), (
            "window with fewer edges than a full prior chunk; rebalance"
        )
        bases[:, ch] = srcs[:, :, p0].min(axis=0)
    # pad srcs with their chunk's base (rel idx 0, coef 0)
    for ch in range(n_chunks):
        p0, p1 = ch * GCH * P, min((ch + 1) * GCH * P, cap)
        blk = srcs[:, :, p0:p1]
        pad = blk < 0
        blk[pad] = np.broadcast_to(bases[None, :, ch, None], blk.shape)[pad]
        rel = blk - bases[None, :, ch, None]
        assert rel.min() >= 0 and rel.max() < IDX_SPAN, (
            f"chunk {ch} span {rel.max()} exceeds int16 gather reach"
        )

    # tile-major layouts [cores, P, n_cols]
    rofs = np.ascontiguousarray(
        rofs_e.reshape(n_cores, n_windows, T, P).transpose(0, 3, 1, 2)
    ).reshape(n_cores, P, n_cols)
    coef = np.ascontiguousarray(
        coef_e.reshape(n_cores, n_windows, T, P).transpose(0, 3, 1, 2)
    ).reshape(n_cores, P, n_cols)

    # wrapped idx16 per (window, chunk): 1024 rel idxs -> 64 int16 cols
    idxcols_per_chunk = GCH * P // 16
    idx16 = np.zeros(
        (n_cores, P, n_windows * n_chunks * idxcols_per_chunk), np.int16
    )
    for c in range(n_cores):
        for w in range(n_windows):
            for ch in range(n_chunks):
                p0, p1 = ch * GCH * P, min((ch + 1) * GCH * P, cap)
                rel = srcs[c, w, p0:p1] - bases[w, ch]
                if p1 - p0 < GCH * P:
                    rel = np.concatenate(
                        [rel, np.zeros(GCH * P - (p1 - p0), np.int64)]
                    )
                cb = (w * n_chunks + ch) * idxcols_per_chunk
                idx16[c, :, cb : cb + idxcols_per_chunk] = _wrap_idx16(rel)

    snode = np.ones((n_cores, P, n_windows), np.float32)
    snode[node_core, node_slot, node_window] = s_all.astype(np.float32)

    out_core = node_core
    out_row = node_window * P + node_slot

    table = (np.asarray(h, np.float64) * (out_deg**-0.5)[:, None]).astype(
        np.float32
    )

    return (
        table, idx16, rofs, coef, snode, bases, out_core, out_row,
        n_windows, T, n_cols,
    )


def _build_nc(N, D, n_windows, T, n_cols, bases):
    import concourse.bacc as bacc
    import concourse.tile as tile
    from concourse import mybir

    f32 = mybir.dt.float32
    bf16 = mybir.dt.bfloat16
    i16 = mybir.dt.int16
    n_chunks = (T + GCH - 1) // GCH
    idxcols_per_chunk = GCH * P // 16
    idxtot = n_windows * n_chunks * idxcols_per_chunk

    # fconst16 layout: rofs | coef | iota | wmat
    f16tot = 2 * n_cols + P + D
    # fconst32 layout: biasf | snode | negr | coef32 | negc
    f32tot = D + n_windows + 3 * n_cols

    nc = bacc.Bacc(
        None, target_bir_lowering=False, debug=False, num_swdge_queues=4
    )
    h_d = nc.declare_dram_parameter("h", [N, D], bf16, isOutput=False)
    idx_d = nc.declare_dram_parameter("idx16", [P, idxtot], i16, isOutput=False)
    fc16_d = nc.declare_dram_parameter("fconst16", [P, f16tot], bf16, isOutput=False)
    fc32_d = nc.declare_dram_parameter("fconst32", [P, f32tot], f32, isOutput=False)
    out_d = nc.declare_dram_parameter("out", [n_windows * P, D], f32, isOutput=True)

    mult = mybir.AluOpType.mult
    AF = mybir.ActivationFunctionType
    DVE_T = T - ACT_T

    with tile.TileContext(nc) as tc:
        with (
            tc.tile_pool(name="singles", bufs=1) as singles,
            tc.tile_pool(name="g", bufs=14) as gpool,
            tc.tile_pool(name="selA", bufs=8) as selApool,
            tc.tile_pool(name="selB", bufs=8) as selBpool,
            tc.tile_pool(name="selC", bufs=5 * ACT_T) as selCpool,
            tc.tile_pool(name="tmpC", bufs=4) as tmpCpool,
            tc.tile_pool(name="agg", bufs=4) as aggpool,
            tc.tile_pool(name="psum", bufs=6, space="PSUM") as psumpool,
            tc.tile_pool(name="psum2", bufs=2, space="PSUM") as psum2pool,
            tc.tile_pool(name="outp", bufs=3) as outpool,
        ):
            # dummy 16-idx gather: pays the gather-ucode IRAM load (~6us)
            # concurrently with the input staging DMAs
            dummy_idx = singles.tile([P, 1], i16)
            nc.gpsimd.memset(dummy_idx[:], 0)
            dummy_g = singles.tile([P, 1, P], bf16)
            nc.gpsimd.dma_gather(
                dummy_g[:],
                h_d[0:IDX_SPAN, :],
                dummy_idx[:],
                16,
                16,
                P,
                single_packet=False,
                queue_num=0,
            )

            idx_sb = singles.tile([P, idxtot], i16)
            hd = min(6 * n_chunks * idxcols_per_chunk, idxtot)
            nc.sync.dma_start(out=idx_sb[:, :hd], in_=idx_d[:, :hd])
            fc16_sb = singles.tile([P, f16tot], bf16)
            nc.sync.dma_start(out=fc16_sb[:], in_=fc16_d[:])
            if hd < idxtot:
                mid = hd + (idxtot - hd) // 2
                nc.sync.dma_start(out=idx_sb[:, hd:mid], in_=idx_d[:, hd:mid])
                nc.sync.dma_start(out=idx_sb[:, mid:], in_=idx_d[:, mid:])
            fc32_sb = singles.tile([P, f32tot], f32)
            nc.sync.dma_start(out=fc32_sb[:], in_=fc32_d[:])

            r_sb = fc16_sb[:, 0:n_cols]
            c_sb = fc16_sb[:, n_cols : 2 * n_cols]
            o0 = 2 * n_cols
            io_sb = fc16_sb[:, o0 : o0 + P]
            w_sb = fc16_sb[:, o0 + P : o0 + P + D]
            b_sb = fc32_sb[:, 0:D]
            s_sb = fc32_sb[:, D : D + n_windows]
            q0 = D + n_windows
            nr_sb = fc32_sb[:, q0 : q0 + n_cols]
            cf_sb = fc32_sb[:, q0 + n_cols : q0 + 2 * n_cols]
            ncf_sb = fc32_sb[:, q0 + 2 * n_cols : q0 + 3 * n_cols]

            qctr = 0
            for w in range(n_windows):
                chunks = []
                for ch in range(n_chunks):
                    nt = min(GCH, T - ch * GCH)
                    g = gpool.tile([P, GCH, P], bf16, tag="g")
                    cb = (w * n_chunks + ch) * idxcols_per_chunk
                    b = int(bases[w, ch])
                    nc.gpsimd.dma_gather(
                        g[:, :nt, :],
                        h_d[b : min(b + IDX_SPAN, N), :],
                        idx_sb[:, cb : cb + nt * (P // 16)],
                        nt * P,
                        nt * P,
                        P,
                        single_packet=False,
                        queue_num=qctr % 4,
                    )
                    qctr += 1
                    chunks.append(g)

                # sel tiles 0..DVE_T-1 on DVE in <=8-tile groups
                sel_dve = []
                t0g = w * T
                offs = 0
                while offs < DVE_T:
                    ng = min(GCH, DVE_T - offs)
                    if ng > 5:
                        sel = selApool.tile([P, GCH, P], bf16, tag="selA")
                    else:
                        sel = selBpool.tile([P, 5, P], bf16, tag="selB")
                    t0 = t0g + offs
                    rb = r_sb[:, t0 : t0 + ng].unsqueeze(2).broadcast_to([P, ng, P])
                    iob = io_sb.unsqueeze(1).broadcast_to([P, ng, P])
                    nc.vector.tensor_tensor(
                        out=sel[:, :ng, :], in0=rb, in1=iob,
                        op=mybir.AluOpType.is_equal,
                    )
                    cb16 = c_sb[:, t0 : t0 + ng].unsqueeze(2).broadcast_to([P, ng, P])
                    nc.vector.tensor_tensor(
                        out=sel[:, :ng, :], in0=sel[:, :ng, :], in1=cb16, op=mult
                    )
                    sel_dve.append((offs, ng, sel))
                    offs += ng

                # sel tiles DVE_T..T-1 on the scalar engine
                sel_act = []
                for j in range(DVE_T, T):
                    t = t0g + j
                    tmp = tmpCpool.tile([P, P], bf16, tag="tmpC")
                    nc.scalar.activation(
                        out=tmp[:], in_=io_sb, func=AF.Abs,
                        bias=nr_sb[:, t : t + 1], scale=1.0,
                    )
                    selc = selCpool.tile([P, P], bf16, tag="selC")
                    nc.scalar.activation(
                        out=selc[:], in_=tmp[:], func=AF.Relu,
                        bias=cf_sb[:, t : t + 1], scale=ncf_sb[:, t : t + 1],
                    )
                    sel_act.append(selc)

                ps = psumpool.tile([P, P], f32)
                for j in range(T):
                    lhsT = chunks[j // GCH][:, j % GCH, :]
                    if j < DVE_T:
                        for offs, ng, sel in sel_dve:
                            if offs <= j < offs + ng:
                                rhs = sel[:, j - offs, :]
                                break
                    else:
                        rhs = sel_act[j - DVE_T][:]
                    nc.tensor.matmul(
                        out=ps[:], lhsT=lhsT, rhs=rhs,
                        start=(j == 0), stop=(j == T - 1),
                    )

                agg = aggpool.tile([P, P], bf16, tag="agg")
                nc.scalar.copy(out=agg[:], in_=ps[:])
                ps2 = psum2pool.tile([P, D], f32)
                nc.tensor.matmul(
                    out=ps2[:], lhsT=agg[:], rhs=w_sb, start=True, stop=True
                )
                o = outpool.tile([P, D], f32)
                nc.vector.tensor_tensor(
                    out=o[:],
                    in0=ps2[:],
                    in1=s_sb[:, w : w + 1].to_broadcast([P, D]),
                    op=mult,
                )
                nc.vector.tensor_add(out=o[:], in0=o[:], in1=b_sb)
                nc.sync.dma_start(out=out_d[w * P : (w + 1) * P, :], in_=o[:])

    nc.compile()
    return nc


def kernel(h, src, dst, distance, weight, bias, _trace=False):
    import ml_dtypes
    from concourse.bass_utils import run_bass_kernel_spmd

    h = np.ascontiguousarray(np.asarray(h, dtype=np.float32))
    weight = np.asarray(weight, dtype=np.float32)
    bias = np.asarray(bias, dtype=np.float32)
    N, D = h.shape

    (
        table, idx16, rofs, coef, snode, bases, out_core, out_row,
        n_windows, T, n_cols,
    ) = _prep_host(h, src, dst, distance, N_CORES)

    bf = ml_dtypes.bfloat16
    table16 = np.ascontiguousarray(table.astype(bf))
    iota = np.broadcast_to(np.arange(P, dtype=np.float32)[None, :], (P, P))
    biasf = np.broadcast_to(bias[None, :], (P, D)).astype(np.float32)

    nc = _build_nc(N, D, n_windows, T, n_cols, bases)

    in_maps = []
    for c in range(N_CORES):
        fconst16 = np.concatenate(
            [rofs[c], coef[c], iota, weight], axis=1
        ).astype(bf)
        fconst32 = np.concatenate(
            [biasf, snode[c], -rofs[c], coef[c], -coef[c]], axis=1
        ).astype(np.float32)
        in_maps.append(
            {
                "h": table16,
                "idx16": np.ascontiguousarray(idx16[c]),
                "fconst16": np.ascontiguousarray(fconst16),
                "fconst32": np.ascontiguousarray(fconst32),
            }
        )

    res = run_bass_kernel_spmd(nc, in_maps, list(range(N_CORES)), trace=_trace)

    stacked = np.stack([res.results[c]["out"] for c in range(N_CORES)])
    out = stacked[out_core, out_row].astype(np.float32)

    if _trace:
        return out, res
    return out


# revision 7
# speedup vs baseline: 1.0168x; 1.0168x over previous
"""DGCN layer (message passing GNN) on 8 Trainium2 NeuronCores via Bass/Tile.

v3. The gather data path is HBM random-read-bandwidth bound (~90 GB/s for
256B rows), so this version minimizes gathered bytes and row-read locality:
  - Single bf16 table (outdeg^-0.5 folded in). No lo/hi table split: each
    window's edges are sorted by src and cut into 1024-edge gather chunks;
    each chunk's dma_gather uses a chunk-specific table base row (shared
    across cores = min over cores), so int16 indices cover the chunk span.
    Sorted rows also improve HBM row-buffer locality.
  - Uniform T=16 tiles/window (balancer keeps every window <= 2048 edges)
    -> exactly 2 x 1024-idx gather instructions per window, 98 total.
  - sel built per window: tiles 0..T-4 in <=8-tile groups on DVE (is_equal
    + coef mult, bf16), last 3 tiles on the Scalar engine via
    Abs(iota - r) then Relu(coef - coef*t) with per-partition bias/scale.
  - 4 SWDGE queues; bf16 matmuls (FWL); interleaved phase 2.
"""

import math

import numpy as np

P = 128
ALPHA = 0.5
N_CORES = 8
GCH = 8  # tiles per dma_gather (hw limit: <=1024 idxs/inst)
ACT_T = 3  # sel tiles per window built on the scalar engine
IDX_SPAN = 32768


def _wrap_idx16(flat):
    """dma_gather index layout: entry k -> partition k%16, column k//16,
    replicated across the 8 gpsimd core groups (partitions 16-127)."""
    n = flat.shape[-1]
    assert n % 16 == 0
    cols = n // 16
    w = np.asarray(flat, np.int16).reshape(cols, 16).T  # [16, cols]
    return np.tile(w, (8, 1))  # [128, cols]


def _prep_host(h, src, dst, distance, n_cores):
    N, D = h.shape
    E = src.shape[0]
    npc = N // n_cores
    n_windows = (npc + P - 1) // P

    src = np.asarray(src).astype(np.int64)
    dst = np.asarray(dst).astype(np.int64)
    distance = np.asarray(distance)

    out_deg = np.bincount(src, minlength=N).astype(np.float64)
    in_deg = np.bincount(dst, minlength=N).astype(np.float64)
    coef_all = (np.float64(ALPHA) ** distance.astype(np.float64)).astype(np.float32)
    s_all = in_deg**-1.5

    # Deal nodes (sorted by in-degree) into n_cores*n_windows bins in rounds;
    # heaviest nodes go to the lightest bins -> per-window edge counts are
    # near-equal, so every window fits the same tile count T.
    n_bins = n_cores * n_windows
    order_nodes = np.argsort(-in_deg, kind="stable")
    node_bin = np.empty(N, np.int64)
    node_slot = np.empty(N, np.int64)
    esum = np.zeros(n_bins, np.int64)
    fill = np.zeros(n_bins, np.int64)
    pos = 0
    while pos < N:
        take = min(n_bins, N - pos)
        nodes_r = order_nodes[pos : pos + take]
        bins_r = np.argsort(esum, kind="stable")[:take]
        node_bin[nodes_r] = bins_r
        node_slot[nodes_r] = fill[bins_r]
        fill[bins_r] += 1
        esum[bins_r] += in_deg[nodes_r].astype(np.int64)
        pos += take
    node_core = node_bin // n_windows
    node_window = node_bin % n_windows

    core_of = node_core[dst]
    w_of = node_window[dst]
    r_of = node_slot[dst].astype(np.float32)

    gw = core_of * n_windows + w_of
    counts = np.bincount(gw, minlength=n_bins)
    maxc = int(counts.max())
    T = max(1, int(math.ceil(maxc / P)))
    cap = T * P
    n_cols = n_windows * T

    # sort edges by (core, window, src) in one argsort
    key = gw * (1 << 17) + src
    order = np.argsort(key, kind="stable")
    sgw = gw[order]
    win_start = np.concatenate([[0], np.cumsum(counts)[:-1]])
    q = np.arange(E, dtype=np.int64) - win_start[sgw]  # pos within window

    core_arr = sgw // n_windows
    w_arr = sgw % n_windows

    # padded per-window edge arrays [cores, windows, cap]
    srcs = np.full((n_cores, n_windows, cap), -1, np.int64)
    rofs_e = np.zeros((n_cores, n_windows, cap), np.float32)
    coef_e = np.zeros((n_cores, n_windows, cap), np.float32)
    srcs[core_arr, w_arr, q] = src[order]
    rofs_e[core_arr, w_arr, q] = r_of[order]
    coef_e[core_arr, w_arr, q] = coef_all[order]

    # chunk bases: global (min over cores) per (window, chunk). Chunks are
    # GCH tiles = GCH*P sorted positions. Position 8ch*128 is always a real
    # edge when counts > previous chunk capacity; assert that.
    n_chunks = (T + GCH - 1) // GCH
    bases = np.zeros((n_windows, n_chunks), np.int64)
    for ch in range(n_chunks):
        p0 = ch * GCH * P
        assert (counts.reshape(n_cores, n_windows) > p0).all(), (
            "window with fewer edges than a full prior chunk; rebalance"
        )
        bases[:, ch] = srcs[:, :, p0].min(axis=0)
    # pad srcs with their chunk's base (rel idx 0, coef 0)
    for ch in range(n_chunks):
        p0, p1 = ch * GCH * P, min((ch + 1) * GCH * P, cap)
        blk = srcs[:, :, p0:p1]
        pad = blk < 0
        blk[pad] = np.broadcast_to(bases[None, :, ch, None], blk.shape)[pad]
        rel = blk - bases[None, :, ch, None]
        assert rel.min() >= 0 and rel.max() < IDX_SPAN, (
            f"chunk {ch} span {rel.max()} exceeds int16 gather reach"
        )

    # tile-major layouts [cores, P, n_cols]
    rofs = np.ascontiguousarray(
        rofs_e.reshape(n_cores, n_windows, T, P).transpose(0, 3, 1, 2)
    ).reshape(n_cores, P, n_cols)
    coef = np.ascontiguousarray(
        coef_e.reshape(n_cores, n_windows, T, P).transpose(0, 3, 1, 2)
    ).reshape(n_cores, P, n_cols)

    # wrapped idx16 per (window, chunk): 1024 rel idxs -> 64 int16 cols
    idxcols_per_chunk = GCH * P // 16
    idx16 = np.zeros(
        (n_cores, P, n_windows * n_chunks * idxcols_per_chunk), np.int16
    )
    for c in range(n_cores):
        for w in range(n_windows):
            for ch in range(n_chunks):
                p0, p1 = ch * GCH * P, min((ch + 1) * GCH * P, cap)
                rel = srcs[c, w, p0:p1] - bases[w, ch]
                if p1 - p0 < GCH * P:
                    rel = np.concatenate(
                        [rel, np.zeros(GCH * P - (p1 - p0), np.int64)]
                    )
                cb = (w * n_chunks + ch) * idxcols_per_chunk
                idx16[c, :, cb : cb + idxcols_per_chunk] = _wrap_idx16(rel)

    snode = np.ones((n_cores, P, n_windows), np.float32)
    snode[node_core, node_slot, node_window] = s_all.astype(np.float32)

    out_core = node_core
    out_row = node_window * P + node_slot

    table = (np.asarray(h, np.float64) * (out_deg**-0.5)[:, None]).astype(
        np.float32
    )

    return (
        table, idx16, rofs, coef, snode, bases, out_core, out_row,
        n_windows, T, n_cols,
    )


def _build_nc(N, D, n_windows, T, n_cols, bases):
    import concourse.bacc as bacc
    import concourse.tile as tile
    from concourse import mybir

    f32 = mybir.dt.float32
    bf16 = mybir.dt.bfloat16
    i16 = mybir.dt.int16
    n_chunks = (T + GCH - 1) // GCH
    idxcols_per_chunk = GCH * P // 16
    idxtot = n_windows * n_chunks * idxcols_per_chunk

    # fconst16 layout: rofs | coef | iota | wmat
    f16tot = 2 * n_cols + P + D
    # fconst32 layout: biasf | snode | negr | coef32 | negc
    f32tot = D + n_windows + 3 * n_cols

    nc = bacc.Bacc(
        None, target_bir_lowering=False, debug=False, num_swdge_queues=4
    )
    h_d = nc.declare_dram_parameter("h", [N, D], bf16, isOutput=False)
    idx_d = nc.declare_dram_parameter("idx16", [P, idxtot], i16, isOutput=False)
    fc16_d = nc.declare_dram_parameter("fconst16", [P, f16tot], bf16, isOutput=False)
    fc32_d = nc.declare_dram_parameter("fconst32", [P, f32tot], f32, isOutput=False)
    out_d = nc.declare_dram_parameter("out", [n_windows * P, D], f32, isOutput=True)

    mult = mybir.AluOpType.mult
    AF = mybir.ActivationFunctionType
    DVE_T = T - ACT_T

    with tile.TileContext(nc) as tc:
        with (
            tc.tile_pool(name="singles", bufs=1) as singles,
            tc.tile_pool(name="g", bufs=14) as gpool,
            tc.tile_pool(name="selA", bufs=8) as selApool,
            tc.tile_pool(name="selB", bufs=8) as selBpool,
            tc.tile_pool(name="selC", bufs=5 * ACT_T) as selCpool,
            tc.tile_pool(name="tmpC", bufs=4) as tmpCpool,
            tc.tile_pool(name="agg", bufs=4) as aggpool,
            tc.tile_pool(name="psum", bufs=6, space="PSUM") as psumpool,
            tc.tile_pool(name="psum2", bufs=2, space="PSUM") as psum2pool,
            tc.tile_pool(name="outp", bufs=3) as outpool,
        ):
            idx_sb = singles.tile([P, idxtot], i16)
            hd = min(6 * n_chunks * idxcols_per_chunk, idxtot)
            nc.sync.dma_start(out=idx_sb[:, :hd], in_=idx_d[:, :hd])
            fc16_sb = singles.tile([P, f16tot], bf16)
            nc.sync.dma_start(out=fc16_sb[:], in_=fc16_d[:])
            if hd < idxtot:
                mid = hd + (idxtot - hd) // 2
                nc.sync.dma_start(out=idx_sb[:, hd:mid], in_=idx_d[:, hd:mid])
                nc.sync.dma_start(out=idx_sb[:, mid:], in_=idx_d[:, mid:])
            fc32_sb = singles.tile([P, f32tot], f32)
            nc.sync.dma_start(out=fc32_sb[:], in_=fc32_d[:])

            r_sb = fc16_sb[:, 0:n_cols]
            c_sb = fc16_sb[:, n_cols : 2 * n_cols]
            o0 = 2 * n_cols
            io_sb = fc16_sb[:, o0 : o0 + P]
            w_sb = fc16_sb[:, o0 + P : o0 + P + D]
            b_sb = fc32_sb[:, 0:D]
            s_sb = fc32_sb[:, D : D + n_windows]
            q0 = D + n_windows
            nr_sb = fc32_sb[:, q0 : q0 + n_cols]
            cf_sb = fc32_sb[:, q0 + n_cols : q0 + 2 * n_cols]
            ncf_sb = fc32_sb[:, q0 + 2 * n_cols : q0 + 3 * n_cols]

            qctr = 0
            for w in range(n_windows):
                chunks = []
                for ch in range(n_chunks):
                    nt = min(GCH, T - ch * GCH)
                    g = gpool.tile([P, GCH, P], bf16, tag="g")
                    cb = (w * n_chunks + ch) * idxcols_per_chunk
                    b = int(bases[w, ch])
                    nc.gpsimd.dma_gather(
                        g[:, :nt, :],
                        h_d[b : min(b + IDX_SPAN, N), :],
                        idx_sb[:, cb : cb + nt * (P // 16)],
                        nt * P,
                        nt * P,
                        P,
                        single_packet=False,
                        queue_num=qctr % 4,
                    )
                    qctr += 1
                    chunks.append(g)

                # sel tiles 0..DVE_T-1 on DVE in <=8-tile groups
                sel_dve = []
                t0g = w * T
                offs = 0
                while offs < DVE_T:
                    ng = min(GCH, DVE_T - offs)
                    if ng > 5:
                        sel = selApool.tile([P, GCH, P], bf16, tag="selA")
                    else:
                        sel = selBpool.tile([P, 5, P], bf16, tag="selB")
                    t0 = t0g + offs
                    rb = r_sb[:, t0 : t0 + ng].unsqueeze(2).broadcast_to([P, ng, P])
                    iob = io_sb.unsqueeze(1).broadcast_to([P, ng, P])
                    nc.vector.tensor_tensor(
                        out=sel[:, :ng, :], in0=rb, in1=iob,
                        op=mybir.AluOpType.is_equal,
                    )
                    cb16 = c_sb[:, t0 : t0 + ng].unsqueeze(2).broadcast_to([P, ng, P])
                    nc.vector.tensor_tensor(
                        out=sel[:, :ng, :], in0=sel[:, :ng, :], in1=cb16, op=mult
                    )
                    sel_dve.append((offs, ng, sel))
                    offs += ng

                # sel tiles DVE_T..T-1 on the scalar engine
                sel_act = []
                for j in range(DVE_T, T):
                    t = t0g + j
                    tmp = tmpCpool.tile([P, P], bf16, tag="tmpC")
                    nc.scalar.activation(
                        out=tmp[:], in_=io_sb, func=AF.Abs,
                        bias=nr_sb[:, t : t + 1], scale=1.0,
                    )
                    selc = selCpool.tile([P, P], bf16, tag="selC")
                    nc.scalar.activation(
                        out=selc[:], in_=tmp[:], func=AF.Relu,
                        bias=cf_sb[:, t : t + 1], scale=ncf_sb[:, t : t + 1],
                    )
                    sel_act.append(selc)

                ps = psumpool.tile([P, P], f32)
                for j in range(T):
                    lhsT = chunks[j // GCH][:, j % GCH, :]
                    if j < DVE_T:
                        for offs, ng, sel in sel_dve:
                            if offs <= j < offs + ng:
                                rhs = sel[:, j - offs, :]
                                break
                    else:
                        rhs = sel_act[j - DVE_T][:]
                    nc.tensor.matmul(
                        out=ps[:], lhsT=lhsT, rhs=rhs,
                        start=(j == 0), stop=(j == T - 1),
                    )

                agg = aggpool.tile([P, P], bf16, tag="agg")
                nc.scalar.copy(out=agg[:], in_=ps[:])
                ps2 = psum2pool.tile([P, D], f32)
                nc.tensor.matmul(
                    out=ps2[:], lhsT=agg[:], rhs=w_sb, start=True, stop=True
                )
                o = outpool.tile([P, D], f32)
                nc.vector.tensor_tensor(
                    out=o[:],
                    in0=ps2[:],
                    in1=s_sb[:, w : w + 1].to_broadcast([P, D]),
                    op=mult,
                )
                nc.vector.tensor_add(out=o[:], in0=o[:], in1=b_sb)
                nc.sync.dma_start(out=out_d[w * P : (w + 1) * P, :], in_=o[:])

    nc.compile()
    return nc


def kernel(h, src, dst, distance, weight, bias, _trace=False):
    import ml_dtypes
    from concourse.bass_utils import run_bass_kernel_spmd

    h = np.ascontiguousarray(np.asarray(h, dtype=np.float32))
    weight = np.asarray(weight, dtype=np.float32)
    bias = np.asarray(bias, dtype=np.float32)
    N, D = h.shape

    (
        table, idx16, rofs, coef, snode, bases, out_core, out_row,
        n_windows, T, n_cols,
    ) = _prep_host(h, src, dst, distance, N_CORES)

    bf = ml_dtypes.bfloat16
    table16 = np.ascontiguousarray(table.astype(bf))
    iota = np.broadcast_to(np.arange(P, dtype=np.float32)[None, :], (P, P))
    biasf = np.broadcast_to(bias[None, :], (P, D)).astype(np.float32)

    nc = _build_nc(N, D, n_windows, T, n_cols, bases)

    in_maps = []
    for c in range(N_CORES):
        fconst16 = np.concatenate(
            [rofs[c], coef[c], iota, weight], axis=1
        ).astype(bf)
        fconst32 = np.concatenate(
            [biasf, snode[c], -rofs[c], coef[c], -coef[c]], axis=1
        ).astype(np.float32)
        in_maps.append(
            {
                "h": table16,
                "idx16": np.ascontiguousarray(idx16[c]),
                "fconst16": np.ascontiguousarray(fconst16),
                "fconst32": np.ascontiguousarray(fconst32),
            }
        )

    res = run_bass_kernel_spmd(nc, in_maps, list(range(N_CORES)), trace=_trace)

    stacked = np.stack([res.results[c]["out"] for c in range(N_CORES)])
    out = stacked[out_core, out_row].astype(np.float32)

    if _trace:
        return out, res
    return out
